# revision 2
# baseline (speedup 1.0000x reference)
"""Axial attention (B,H,W,C)=(8,128,128,256), 8 heads, for 8 trn2 NeuronCores.

Sharding: data-parallel over batch B=8 -> one batch element per core.
Per core, two passes over x[b]:
  phase A: attention along H (one sequence per column w), writes
           oh + bout0 + bout1 to a bf16 HBM scratch in (H,W,C) layout.
  phase B: attention along W (one sequence per row h), adds the scratch row
           and writes the final fp32 output row.

v2 design notes (per group of G=4 sequences, T=128 tokens, C=256, 8 heads):
  * QKV projections run as fp8e4m3 DoubleRow matmuls with an exact hi/lo
    split (x = xh + xl, w = wh + wl, dropping only the xl*wl term): 3
    DoubleRow instructions replace the 2 bf16 contraction-chunk matmuls at
    0.75x PE cost and bf16-level accuracy. Weights are pre-scaled by 16 on
    the host so the lo residuals stay out of fp8 subnormal range; the 16x
    factors are unwound for free via the exp scale (2^-8) and the V-copy
    scale (2^-4).
  * Scores stay bf16, computed transposed (keys on partitions) with the
    fused ones-column-in-V softmax-denominator trick; no max subtraction
    (scores are O(1): Wq carries e^-0.5).
  * The onorm -> out-projection transpose uses the DMA xbar
    (dma_start_transpose) instead of PE transposes, freeing both the PE
    rows and the PSUM->SBUF copy.
  * PSUM is one pool of 4 rotating [128,1024]f32 slots (2 banks each);
    Q/K/V psums, 4 score tiles, 2 AV accumulators and the out-projection
    accumulator cycle through it, which lets every PSUM->SBUF move be a
    single big instruction: castQ/castK/final-add on DVE, castV and the 4
    exps on Act. Engine budget/group: PE ~4.3us, DVE ~5.4us, Act ~5.2us.

Toolchain note: this neuronxcc accepts at most ONE sync-wait per
instruction; Tile's multi-wait sync is legalized post-scheduling by
hoisting extra waits onto same-engine nops.
"""

import sys

sys.path.insert(0, "/opt/trn_rl_repo")

import numpy as np
import ml_dtypes

import concourse.bass as bass
import concourse.tile as tile
from concourse import mybir
from concourse.bass_utils import run_bass_kernel_spmd
from concourse.vector_clock import ScopedClock

F32 = mybir.dt.float32
BF16 = mybir.dt.bfloat16
FP8 = mybir.dt.float8e4
AF = mybir.ActivationFunctionType
OP = mybir.AluOpType
DR = mybir.MatmulPerfMode.DoubleRow

H = 128
W = 128
C = 256
HEADS = 8
E = C // HEADS  # 32
T = 128  # sequence length for both axes
G = 4  # sequences processed per group (batched projections)

# --- workaround: this toolchain's codegen accepts at most ONE sync-wait per
# instruction; redistribute extra waits onto preceding same-engine nops. ---

_MAXW = 1


def _patched_drain_and_barrier(self, tick_clock, wait_clock):
    probe = self.nc.sync.nop(nofuse=True)
    wait_clock.add_sem_waits(probe.ins, ScopedClock({None: tick_clock.global_clock}))
    conds = list(probe.ins.sync_info.on_wait)
    probe.ins.sync_info.on_wait = conds[:_MAXW]
    rest = conds[_MAXW:]
    while rest:
        n2 = self.nc.sync.nop(nofuse=True)
        if n2.ins.sync_info is None:
            n2.ins.sync_info = mybir.SyncInfo(on_wait=[], on_update=[])
        n2.ins.sync_info.on_wait = rest[:_MAXW]
        rest = rest[_MAXW:]
    self.nc.sync.drain()
    self.nc.all_engine_barrier()
    popped = self.nc._tile_sem_poison_stack.pop()
    assert popped is self._sem_poison
    self.nc.clear_and_free_semaphores(list(self.sems.allocated().values()))
    self.nc.all_engine_barrier()


tile.TileContext._drain_and_barrier = _patched_drain_and_barrier


_CTRL_OPS = ("InstNoOp", "InstDrain", "InstEventSemaphore", "InstCompareAndBranch")


def _split_waits(nc, limit=_MAXW, compute_limit=1):
    """Hoist extra sync-waits onto fresh nops directly before their owner."""
    n_split = 0
    for fn in nc.m.functions:
        for blk in fn.blocks:
            insts = blk.instructions
            out = []
            for inst in insts:
                si = inst.sync_info
                limit = (
                    _MAXW if type(inst).__name__ in _CTRL_OPS else compute_limit
                )
                if si is not None and len(si.on_wait) > limit:
                    waits = list(si.on_wait)
                    extra, keep = waits[:-limit], waits[-limit:]
                    k = 0
                    while extra:
                        nop = mybir.InstNoOp(
                            name=f"{inst.name}-wsplit{k}",
                            engine=inst.engine,
                            bass_nofuse=True,
                            sync_info=mybir.SyncInfo(
                                on_wait=extra[:limit], on_update=[]
                            ),
                        )
                        nc.register_instruction(nop, overwrite=True)
                        out.append(nop)
                        extra = extra[limit:]
                        k += 1
                        n_split += 1
                    si.on_wait = keep
                out.append(inst)
            blk.instructions = out
    return n_split


def _bcast_rows(handle_ap, rows):
    """AP that broadcasts a 1D dram tensor across `rows` partitions."""
    return bass.AP(
        tensor=handle_ap.tensor,
        offset=handle_ap.offset,
        ap=[[0, rows]] + [list(p) for p in handle_ap.ap],
    )


def _ap(base, offset, dims):
    """Raw AP relative to an existing AP's tensor/offset."""
    return bass.AP(tensor=base.tensor, offset=base.offset + offset, ap=dims)


def _build():
    nc = bass.Bass("TRN2", target_bir_lowering=False, debug=False)

    NG = W // G  # 32 groups per pass

    # host-packed fp8 hi/lo inputs: [group, p, hl, kk, s*T+t] where the
    # contracted channel is c = kk*128 + p
    xta = nc.dram_tensor("xta", [NG, 128, 2, 2, G * T], FP8, kind="ExternalInput")
    xtc = nc.dram_tensor("xtc", [NG, 128, 2, 2, G * T], FP8, kind="ExternalInput")
    # packed fp8 hi/lo weights: [p, hl, kk, 3C] (Q cols pre-scaled e^-0.5*16,
    # K,V cols pre-scaled 16)
    w8a = nc.dram_tensor("w8a", [128, 2, 2, 3 * C], FP8, kind="ExternalInput")
    w8c = nc.dram_tensor("w8c", [128, 2, 2, 3 * C], FP8, kind="ExternalInput")
    wout0 = nc.dram_tensor("wout0", [C, C], BF16, kind="ExternalInput")
    wout1 = nc.dram_tensor("wout1", [C, C], BF16, kind="ExternalInput")
    bsum = nc.dram_tensor("bsum", [C], F32, kind="ExternalInput")
    out = nc.dram_tensor("out", [H, W, C], F32, kind="ExternalOutput")
    scratch = nc.dram_tensor("ohs", [H, W, C], BF16)

    xta_ap = xta.ap()
    xtc_ap = xtc.ap()
    out_ap = out.ap()
    sc_ap = scratch.ap()

    with tile.TileContext(nc) as tc:
        with (
            tc.tile_pool(name="const", bufs=1) as const,
            tc.tile_pool(name="work", bufs=4) as work,
            tc.tile_pool(name="ebp", bufs=6) as ebp,
            tc.tile_pool(name="xp", bufs=3) as xp,
            tc.tile_pool(name="ps", bufs=4, space="PSUM") as ps,
        ):
            # ---- constants ----
            bsum_sb = const.tile([128, C], F32, tag="bsum")
            nc.gpsimd.dma_start(out=bsum_sb, in_=_bcast_rows(bsum.ap(), 128))

            w8_sb = {}
            wout_sb = {}
            for ax, (w8_d, wout_d) in enumerate([(w8a, wout0), (w8c, wout1)]):
                t8 = const.tile([128, 2, 2, 3 * C], FP8, tag=f"w8{ax}")
                nc.gpsimd.dma_start(out=t8, in_=w8_d.ap())
                w8_sb[ax] = t8
                wo2 = wout_d.ap().rearrange("(k p) n -> k p n", p=128)
                for k in range(2):
                    t_o = const.tile([128, C], BF16, tag=f"wout{ax}{k}")
                    nc.gpsimd.dma_start(out=t_o, in_=wo2[k])
                    wout_sb[ax, k] = t_o

            # persistent V buffers [128, s, (h, E+1)] with ones columns
            NVP = 3
            vp_bufs = []
            for i in range(NVP):
                vpb = const.tile([128, G, HEADS * (E + 1)], BF16, tag=f"vp{i}")
                nc.gpsimd.memset(vpb, 1.0)
                vp_bufs.append(vpb)

            def axial_pass(ax, n_groups=NG):
                """ax=0: sequences along H (fixed w). ax=1: along W (fixed h)."""
                xt_ap = xta_ap if ax == 0 else xtc_ap
                w8t = w8_sb[ax]

                for grp in range(n_groups):
                    j0 = grp * G

                    if ax == 1:
                        ohrow = work.tile([128, G, C], BF16, tag="ohrow")
                        nc.sync.dma_start(
                            out=ohrow,
                            in_=sc_ap[j0 : j0 + G].rearrange("h w c -> w h c"),
                        )
                        og = work.tile([128, G, C], F32, tag="og")
                    else:
                        og = work.tile([128, G, C], BF16, tag="oa")

                    # ---- x tile (fp8 hi/lo, DoubleRow layout) ----
                    xt = xp.tile([128, 2, 2, G * T], FP8, tag="xt")
                    nc.sync.dma_start(out=xt, in_=xt_ap[grp])
                    xh = xt[:, 0]  # [128, 2, 512]
                    xl = xt[:, 1]

                    # ---- Q/K projections: out[c', tok], 2 m-chunks each ----
                    # hi/lo fp8 DoubleRow: wh.xh + wl.xh + wh.xl
                    qk_sb = []
                    for which in range(2):  # 0: Q, 1: K
                        pp = ps.tile([128, 2 * G * T], F32, tag="ps")
                        for m in range(2):
                            n0 = which * C + m * 128
                            dst = pp[:, m * G * T : (m + 1) * G * T]
                            terms = (
                                (0, xh, False),  # wh . xh
                                (1, xh, False),  # wl . xh
                                (0, xl, True),  # wh . xl (stop)
                            )
                            for i, (hl, xop, last) in enumerate(terms):
                                nc.tensor.matmul(
                                    dst,
                                    w8t[:, hl, :, n0 : n0 + 128],
                                    xop,
                                    start=(i == 0),
                                    stop=last,
                                    perf_mode=DR,
                                )
                        sb = work.tile([128, 2 * G * T], BF16, tag=f"qk{which}")
                        nc.vector.tensor_copy(out=sb, in_=pp)
                        qk_sb.append(sb)
                    qtb, ktb = qk_sb

                    # ---- V projection: out[tok, c], 4 seqs ----
                    pv = ps.tile([128, G * C], F32, tag="ps")
                    for s in range(G):
                        dst = pv[:, s * C : (s + 1) * C]
                        terms = (
                            (0, xh, False),
                            (1, xh, False),
                            (0, xl, True),
                        )
                        for i, (hl, xop, last) in enumerate(terms):
                            nc.tensor.matmul(
                                dst,
                                xop[:, :, s * T : (s + 1) * T],
                                w8t[:, hl, :, 2 * C : 3 * C],
                                start=(i == 0),
                                stop=last,
                                perf_mode=DR,
                            )
                    vp = vp_bufs[grp % NVP]
                    # vp[:, s, h*(E+1)+e] = pv[:, s, h, e] * 2^-4
                    nc.scalar.activation(
                        out=_ap(
                            vp[:],
                            0,
                            [list(vp.ap[0]), [HEADS * (E + 1), G], [E + 1, HEADS], [1, E]],
                        ),
                        in_=pv.rearrange("p (s h e) -> p s h e", h=HEADS, e=E),
                        func=AF.Copy,
                        scale=2.0**-4,
                    )

                    # ---- scores (bf16, keys on partitions) + exp ----
                    # sc tile [128, (q2, s4, t128)] per (hg, qpair)
                    ebq = {}
                    for hg in range(2):
                        for qp in range(2):
                            sct = ps.tile([128, 2 * G * T], F32, tag="ps")
                            for qq in range(2):
                                q = qp * 2 + qq
                                off = q * E
                                for s in range(G):
                                    nc.tensor.matmul(
                                        sct[
                                            :,
                                            qq * G * T
                                            + s * T : qq * G * T
                                            + (s + 1) * T,
                                        ],
                                        ktb[
                                            off : off + E,
                                            hg * G * T + s * T : hg * G * T + (s + 1) * T,
                                        ],
                                        qtb[
                                            off : off + E,
                                            hg * G * T + s * T : hg * G * T + (s + 1) * T,
                                        ],
                                        start=True,
                                        stop=True,
                                        tile_position=(off, 0),
                                    )
                            eb = ebp.tile([128, 2 * G * T], BF16, tag="eb")
                            nc.scalar.activation(
                                out=eb, in_=sct, func=AF.Exp, scale=2.0**-8
                            )
                            ebq[hg, qp] = eb
                        # AV for this hg right after its exps
                        if hg == 0:
                            po0 = ps.tile([128, 2 * G * T], F32, tag="ps")
                        else:
                            po1 = ps.tile([128, 2 * G * T], F32, tag="ps")
                    # (allocation order fixed: sc00,sc01,po0? no: emitted below)

                    # NOTE: allocation of po0/po1 happened interleaved above to
                    # keep the psum slot rotation aligned with write order.
                    onorm_l = []
                    den_l = []
                    for hg, po in ((0, po0), (1, po1)):
                        for sp in range(2):
                            for si in range(2):
                                s = sp * 2 + si
                                for q in range(4):
                                    qp, qq = divmod(q, 2)
                                    nc.tensor.matmul(
                                        po[
                                            :,
                                            s * 4 * (E + 1)
                                            + q * (E + 1) : s * 4 * (E + 1)
                                            + (q + 1) * (E + 1),
                                        ],
                                        ebq[hg, qp][
                                            :, qq * G * T + s * T : qq * G * T + (s + 1) * T
                                        ],
                                        vp[
                                            :,
                                            s,
                                            (hg * 4 + q) * (E + 1) : (hg * 4 + q + 1)
                                            * (E + 1),
                                        ],
                                        start=True,
                                        stop=True,
                                    )
                        # reciprocal of the 16 denominators of this hg
                        den = work.tile([128, 4, 4], F32, tag="den")
                        nc.vector.reciprocal(
                            out=den,
                            in_=_ap(
                                po[:],
                                E,
                                [list(po.ap[0]), [4 * (E + 1), G], [E + 1, 4]],
                            ),
                        )
                        den_l.append(den)
                        # normalize: onorm[tok, (s, q*32+e)] bf16
                        onorm = work.tile([128, G, 4 * E], BF16, tag=f"onorm{hg}")
                        for sp in range(2):
                            o4 = _ap(
                                po[:],
                                sp * 2 * 4 * (E + 1),
                                [
                                    list(po.ap[0]),
                                    [4 * (E + 1), 2],
                                    [E + 1, 4],
                                    [1, E],
                                ],
                            )
                            rb = _ap(
                                den[:],
                                sp * 2 * 4,
                                [list(den.ap[0]), [4, 2], [1, 4], [0, E]],
                            )
                            nc.vector.tensor_tensor(
                                out=onorm[:, sp * 2 : sp * 2 + 2, :].rearrange(
                                    "p s (q e) -> p s q e", e=E
                                ),
                                in0=o4,
                                in1=rb,
                                op=OP.mult,
                            )
                        onorm_l.append(onorm)

                    # ---- transpose via DMA xbar: otb[c', s, tok] per hg ----
                    otb_l = []
                    for hg in range(2):
                        otb = work.tile([128, G, T], BF16, tag=f"otb{hg}")
                        nc.sync.dma_start_transpose(
                            out=otb, in_=onorm_l[hg].rearrange("p s c -> p (s c)")
                        )
                        otb_l.append(otb)

                    # ---- out projection (bf16) ----
                    fps = ps.tile([128, G * C], F32, tag="ps")
                    for s in range(G):
                        for hg in range(2):
                            nc.tensor.matmul(
                                fps[:, s * C : (s + 1) * C],
                                otb_l[hg][:, s, :],
                                wout_sb[ax, hg],
                                start=(hg == 0),
                                stop=(hg == 1),
                            )

                    # ---- final add + store ----
                    if ax == 0:
                        bs = bsum_sb[:]
                        in1 = bass.AP(
                            tensor=bs.tensor,
                            offset=bs.offset,
                            ap=[list(bs.ap[0]), [0, G], list(bs.ap[1])],
                        )
                    else:
                        in1 = ohrow[:]
                    nc.vector.tensor_tensor(
                        out=og,
                        in0=fps.rearrange("p (s c) -> p s c", c=C),
                        in1=in1,
                        op=OP.add,
                    )

                    if ax == 0:
                        nc.sync.dma_start(out=sc_ap[:, j0 : j0 + G, :], in_=og)
                    else:
                        nc.sync.dma_start(
                            out=out_ap[j0 : j0 + G].rearrange("h w c -> w h c"),
                            in_=og,
                        )

            axial_pass(0)
            axial_pass(1)

    _split_waits(nc)
    return nc


_NC = None


def _get_nc():
    global _NC
    if _NC is None:
        _NC = _build()
    return _NC


def _fp8_hilo(a):
    """Split float32 array into exact-ish fp8e4m3 hi/lo pair, stacked on a
    new axis 0."""
    f8 = ml_dtypes.float8_e4m3fn
    hi = a.astype(f8)
    lo = (a - hi.astype(np.float32)).astype(f8)
    return np.stack([hi, lo])


def _pack_w(wcat):
    """[C, 3C] f32 -> [128, 2(hl), 2(kk), 3C] fp8."""
    hl = _fp8_hilo(wcat)  # [2, C, 3C]
    return np.ascontiguousarray(
        hl.reshape(2, 2, 128, 3 * C).transpose(2, 0, 1, 3)
    )


def _pack_x(xg):
    """[NG, C, G*T] f32 -> [NG, 128, 2(hl), 2(kk), G*T] fp8."""
    hl = _fp8_hilo(xg)  # [2, NG, C, G*T]
    ng = xg.shape[0]
    return np.ascontiguousarray(
        hl.reshape(2, ng, 2, 128, G * T).transpose(1, 3, 0, 2, 4)
    )


def make_in_maps(x, Wq0, Wkv0, Wout0, bout0, Wq1, Wkv1, Wout1, bout1):
    bf = ml_dtypes.bfloat16
    scale = float(E) ** -0.5
    SC = 16.0
    w0 = np.concatenate([Wq0 * (scale * SC), Wkv0 * SC], axis=1).astype(np.float32)
    w1 = np.concatenate([Wq1 * (scale * SC), Wkv1 * SC], axis=1).astype(np.float32)
    xf = np.asarray(x, dtype=np.float32)
    shared = {
        "w8a": _pack_w(w0),
        "w8c": _pack_w(w1),
        "wout0": np.asarray(Wout0, dtype=bf),
        "wout1": np.asarray(Wout1, dtype=bf),
        "bsum": np.asarray(bout0 + bout1, dtype=np.float32),
    }
    NG = W // G
    maps = []
    for b in range(x.shape[0]):
        e = xf[b]  # (H, W, C)
        # xta[g, c, s*T+h] = x[h, 4g+s, c]  (phase A: sequences along H)
        xta_f = np.ascontiguousarray(
            e.transpose(1, 2, 0).reshape(NG, G, C, H).transpose(0, 2, 1, 3)
        ).reshape(NG, C, G * T)
        # xtc[g, c, s*T+w] = x[4g+s, w, c]  (phase B: sequences along W)
        xtc_f = np.ascontiguousarray(
            e.reshape(H // G, G, W, C).transpose(0, 3, 1, 2)
        ).reshape(NG, C, G * T)
        maps.append({"xta": _pack_x(xta_f), "xtc": _pack_x(xtc_f), **shared})
    return maps


def kernel(x, Wq0, Wkv0, Wout0, bout0, Wq1, Wkv1, Wout1, bout1):
    nc = _get_nc()
    in_maps = make_in_maps(
        np.asarray(x),
        np.asarray(Wq0),
        np.asarray(Wkv0),
        np.asarray(Wout0),
        np.asarray(bout0, dtype=np.float32),
        np.asarray(Wq1),
        np.asarray(Wkv1),
        np.asarray(Wout1),
        np.asarray(bout1, dtype=np.float32),
    )
    res = run_bass_kernel_spmd(nc, in_maps, core_ids=list(range(8)))
    return np.stack([r["out"] for r in res.results]).astype(np.float32)
